# revision 1
# baseline (speedup 1.0000x reference)
"""MoE routed matmul on 8 NeuronCores (Trainium2, Bass).

Problem: out[b] = x[b] @ W[idx[b]]  with  x:(2048,256), W:(64,256,256),
idx:(2048,1) int32.

Strategy: expert-parallel. Experts (contexts) are sharded 8-per-core.
The host routes samples to the core that owns their expert (this is the
all-to-all, done during input sharding), padding each expert's sample
group to a fixed capacity CAP so the SPMD device program is fully
static. Each core then does 8 dense (CAP x 256) @ (256 x 256) matmuls —
weights are read from HBM exactly once across the whole device, which is
what the memory-bound roofline wants. The host scatters the device
output back to the original sample order.

Device program per core (raw Bass, manual semaphores):
  sync   : DMA x^T (1 tile) + expert weight tiles into SBUF
  tensor : per expert, 2 accumulating matmuls (K=256 split in 2) into a
           PSUM half-bank; expert pairs share a bank at partition
           offsets 0/64 so downstream copies/DMAs are full 128-wide
  vector : PSUM -> SBUF copy per expert pair (unless psum_direct)
  scalar : DMA each pair's (128, 256) result back to DRAM

niter > 1 replicates the body with double-buffered inputs and WAR
semaphore chaining — used by the benchmark harness to measure
steady-state per-iteration HW time via wall-clock slope.
"""

import numpy as np
from contextlib import ExitStack

B, D, U, C = 2048, 256, 256, 64
NCORES = 8
EPC = C // NCORES  # experts per core
CAP = 64           # per-expert sample capacity (padded)

_prog_cache: dict = {}


def _build_program(cap: int, niter: int = 1, wgroup: int = 1, warmup: int = 0,
                   serial: bool = False, swap: bool = False, stage: str = "full",
                   dualring: bool = False):
    import concourse.bass as bass
    from concourse import mybir
    from concourse.bass import compact_to_ranges

    f32 = mybir.dt.float32
    assert cap % 2 == 0 and (2 * cap) % 32 == 0
    assert EPC % wgroup == 0
    npair = EPC // 2
    ngrp = EPC // wgroup
    nc = bass.Bass()
    # xt: x^T with samples grouped by expert, [D, EPC*cap]
    xt = nc.declare_dram_parameter("xt", [D, EPC * cap], f32, isOutput=False)
    w = nc.declare_dram_parameter("w", [EPC, D, U], f32, isOutput=False)
    if swap:
        # transposed output: per pair, [u-row 128, (e0u0,e0u1,e1u0,e1u1), cap]
        out = nc.declare_dram_parameter("out", [EPC // 2, 128, 4, cap], f32,
                                        isOutput=True)
    else:
        out = nc.declare_dram_parameter("out", [EPC * cap, U], f32, isOutput=True)

    NSET = 2 if niter > 1 else 1

    with ExitStack() as ctx:
        # xt SBUF: [128, 2, EPC*cap] — the two K-chunks in a free dim
        sb_xt = [
            ctx.enter_context(nc.sbuf_tensor(f"sb_xt{s}", [128, 2, EPC * cap], f32))
            for s in range(NSET)
        ]
        # w SBUF per DMA group: [128, wgroup, 2, U]
        sb_w = [
            [
                ctx.enter_context(
                    nc.sbuf_tensor(f"sb_w{g}_{s}", [128, wgroup, 2, U], f32)
                )
                for s in range(NSET)
            ]
            for g in range(ngrp)
        ]
        sb_out = [
            ctx.enter_context(nc.sbuf_tensor(f"sb_out{p}", [128, U], f32))
            for p in range(npair)
        ]
        # one full PSUM bank per expert pair
        ps = [
            ctx.enter_context(nc.psum_tensor(f"ps{p}", [128, 512], f32))
            for p in range(npair)
        ]
        if warmup:
            sb_warm = ctx.enter_context(nc.sbuf_tensor("sb_warm", [128, 512], f32))
            ps_warm = ctx.enter_context(nc.psum_tensor("ps_warm", [128, 512], f32))

        # Dedicated sems per buffer group: a wait threshold on a sem that
        # counts several in-flight DMAs is unsound (a DMA's +16 completion
        # is split +1 across 16 SDMA engines, so a later DMA's increments
        # can satisfy an earlier DMA's threshold while it still has a
        # straggler engine). One sem per buffer makes thresholds exact.
        warm_sem = ctx.enter_context(nc.semaphore("warm_sem"))
        xt_sem = ctx.enter_context(nc.semaphore("xt_sem"))
        w_sem = [ctx.enter_context(nc.semaphore(f"w_sem{g}")) for g in range(ngrp)]
        mm_sem = ctx.enter_context(nc.semaphore("mm_sem"))
        cp_sem = ctx.enter_context(nc.semaphore("cp_sem"))
        out_sem = [ctx.enter_context(nc.semaphore(f"out_sem{p}")) for p in range(npair)]

        # Semaphores are NOT cleared when a loaded NEFF is re-executed, so
        # absolute wait thresholds would be stale on the second run. Clear
        # the whole kernel sem range up front (same preamble the BIR
        # lowering path emits), then a pseudo-sync barrier keeps every
        # engine parked until the clears retire.
        for sem_range in compact_to_ranges(
            [s for s in nc._kernel_sem_range if s not in nc.barrier_sems]
        ):
            nc.gpsimd.dma_reset(sem_range)
            nc.gpsimd.sem_clear(sem_range)
        nc._nrt_pseudo_barrier()
        if warmup:
            # Zero a scratch tile for PE warmup matmuls (gpsimd is idle).
            nc.gpsimd.memset(sb_warm[:, :], 0.0)
            nc.gpsimd.sem_inc(warm_sem, 1)

        block = ctx.enter_context(nc.Block())

        # DRAM access patterns with both K-chunks in the free dim
        xt_r = xt.rearrange("(k p) c -> p k c", k=2)        # [128, 2, EPC*cap]
        w_r = [
            w[g * wgroup:(g + 1) * wgroup].rearrange("e (k p) u -> p e k u", k=2)
            for g in range(ngrp)
        ]

        @block.sync
        def _(sync):
            for i in range(niter):
                s = i % NSET
                if serial and i >= 1:
                    # benchmark mode: no cross-iteration overlap, so each
                    # iteration behaves like an isolated cold call
                    if stage == "dma":
                        sync.wait_ge(w_sem[ngrp - 1], 16 * i)
                    elif stage == "dmamm":
                        sync.wait_ge(mm_sem, 8 * i)
                    else:
                        for p in range(npair):
                            sync.wait_ge(out_sem[p], 16 * i)
                if i >= 2:
                    # xt set s was read by all matmuls of iter i-2
                    sync.wait_ge(mm_sem, 8 * (i - 1))
                sync.dma_start(sb_xt[s][:, :, :], xt_r).then_inc(xt_sem, 16)
                for g in range(ngrp):
                    if dualring and g >= ngrp // 2:
                        continue  # issued from the vector engine's ring
                    if i >= 2:
                        # last expert of group g, iter i-2, done
                        sync.wait_ge(mm_sem, 8 * (i - 2) + (g + 1) * wgroup)
                    sync.dma_start(sb_w[g][s][:, :, :, :], w_r[g]).then_inc(w_sem[g], 16)
            if stage == "dma":
                # stripped bench variant: nothing downstream consumes the
                # input sems, so quiesce the DMAs before the program ends
                sync.wait_ge(xt_sem, 16 * niter)
                for g in range(ngrp):
                    sync.wait_ge(w_sem[g], 16 * niter)

        @block.tensor
        def _(tensor):
            if stage == "dma":
                return
            if warmup:
                tensor.wait_ge(warm_sem, 1)
            for i in range(niter):
                if warmup:
                    # Dummy matmuls: sustained PE activity releases the HAM
                    # clock gate (1.2 -> 2.4 GHz) while input DMAs stream, so
                    # the real matmuls run at full rate even in a cold call.
                    # (Inside the loop so serial-mode benches see the cold-
                    # call behaviour; the shipped kernel has niter=1.)
                    for _ in range(warmup):
                        tensor.matmul(
                            ps_warm[:, :], sb_warm[:, 0:128], sb_warm[:, :],
                            start=True, stop=True,
                        )
                s = i % NSET
                for j in range(EPC):
                    p, half = j // 2, j % 2
                    g, e_local = j // wgroup, j % wgroup
                    if j == 0:
                        tensor.wait_ge(xt_sem, 16 * (i + 1))
                    if e_local == 0:
                        tensor.wait_ge(w_sem[g], 16 * (i + 1))
                    if i >= 1 and stage == "full":
                        # pair bank p was copied out during iter i-1
                        tensor.wait_ge(cp_sem, npair * (i - 1) + p + 1)
                    if swap:
                        # W stationary (full 128-wide), x streams: half the
                        # streamed rows of the x-stationary layout. Output
                        # lands transposed; the host fixes that up.
                        for h in range(2):
                            q = half * 2 + h
                            for k in range(2):
                                mm = tensor.matmul(
                                    ps[p][:, q * cap:(q + 1) * cap],
                                    sb_w[g][s][:, e_local, k, h * 128:(h + 1) * 128],
                                    sb_xt[s][:, k, j * cap:(j + 1) * cap],
                                    start=(k == 0),
                                    stop=(k == 1),
                                )
                    else:
                        for k in range(2):
                            mm = tensor.matmul(
                                ps[p][half * cap:(half + 1) * cap, 0:U],
                                sb_xt[s][:, k, j * cap:(j + 1) * cap],
                                sb_w[g][s][:, e_local, k, :],
                                start=(k == 0),
                                stop=(k == 1),
                            )
                    mm.then_inc(mm_sem, 1)

        if dualring:
            @block.gpsimd
            def _(gpsimd):
                for i in range(niter):
                    s = i % NSET
                    for g in range(ngrp // 2, ngrp):
                        if i >= 2:
                            gpsimd.wait_ge(mm_sem, 8 * (i - 2) + (g + 1) * wgroup)
                        gpsimd.dma_start(
                            sb_w[g][s][:, :, :, :], w_r[g]
                        ).then_inc(w_sem[g], 16)

        @block.vector
        def _(vector):
            if stage in ("dma", "dmamm"):
                return
            for i in range(niter):
                for p in range(npair):
                    vector.wait_ge(mm_sem, 8 * i + 2 * p + 2)
                    if i >= 1:
                        vector.wait_ge(out_sem[p], 16 * i)
                    vector.tensor_copy(
                        sb_out[p][:, :], ps[p][:, 0:4 * cap if swap else U]
                    ).then_inc(cp_sem, 1)

        @block.scalar
        def _(scalar):
            if stage in ("dma", "dmamm"):
                return
            for i in range(niter):
                for p in range(npair):
                    scalar.wait_ge(cp_sem, npair * i + p + 1)
                    if swap:
                        dst = out[p].rearrange("a b c -> a (b c)")
                    else:
                        dst = out[p * 2 * cap:(p + 1) * 2 * cap, :]
                    scalar.dma_start(dst, sb_out[p][:, :]).then_inc(out_sem[p], 16)
            for p in range(npair):
                scalar.wait_ge(out_sem[p], 16 * niter)

    return nc


def _route(content_idx: np.ndarray, x: np.ndarray, cap: int):
    """Sort samples by expert; compute per-core padded x^T shards."""
    idx = content_idx.reshape(-1).astype(np.int64)
    order = np.argsort(idx, kind="stable")
    e_sorted = idx[order]
    counts = np.bincount(idx, minlength=C)
    while counts.max() > cap:
        cap *= 2
    start = np.zeros(C, dtype=np.int64)
    start[1:] = np.cumsum(counts)[:-1]
    slot = np.arange(B) - start[e_sorted]
    core = e_sorted // EPC
    col = (e_sorted % EPC) * cap + slot

    xt_all = np.zeros((NCORES, D, EPC * cap), dtype=np.float32)
    xt_all[core, :, col] = x[order]
    return cap, order, core, col, xt_all


def _unshard(outs: np.ndarray, order, core, col, cap: int, swap: bool) -> np.ndarray:
    """Scatter per-core padded device output back to original sample order."""
    out_full = np.empty((B, U), dtype=np.float32)
    if not swap:
        out_full[order] = outs[core, col, :]
    else:
        npair = EPC // 2
        a = outs.reshape(NCORES, npair, 128, 2, 2, cap)  # (c, p, r, e, h, i)
        a = a.transpose(0, 1, 3, 4, 2, 5)                # (c, p, e, h, r, i)
        a = a.reshape(NCORES, EPC, U, cap)
        out_full[order] = a[core, col // cap, :, col % cap]
    return out_full


def _make_in_maps(xt_all: np.ndarray, kernel_w: np.ndarray):
    w = np.ascontiguousarray(kernel_w.reshape(NCORES, EPC, D, U), dtype=np.float32)
    return [{"xt": xt_all[c], "w": w[c]} for c in range(NCORES)]


def kernel(content_idx: np.ndarray, x: np.ndarray, kernel: np.ndarray) -> np.ndarray:
    from concourse.bass_utils import run_bass_kernel_spmd

    cap, order, core, col, xt_all = _route(content_idx, x, CAP)
    if cap > CAP:
        # Pathologically skewed routing (an expert holds >CAP samples) can't
        # use the static pair-packed program. Unreachable for the fixed-seed
        # problem data; fall back to a host computation to stay correct.
        idx = content_idx.reshape(-1).astype(np.int64)
        return np.einsum("bd,bdu->bu", x.astype(np.float32),
                         kernel.astype(np.float32)[idx]).astype(np.float32)

    key = (cap, 1)
    if key not in _prog_cache:
        _prog_cache[key] = _build_program(cap, 1)
    nc = _prog_cache[key]

    in_maps = _make_in_maps(xt_all, kernel)
    res = run_bass_kernel_spmd(nc, in_maps, list(range(NCORES)))
    outs = np.stack([res.results[c]["out"] for c in range(NCORES)])
    return _unshard(outs, order, core, col, cap, swap=False)



# revision 2
# speedup vs baseline: 1.1734x; 1.1734x over previous
"""MoE routed matmul on 8 NeuronCores (Trainium2, Bass).

Problem: out[b] = x[b] @ W[idx[b]]  with  x:(2048,256), W:(64,256,256),
idx:(2048,1) int32.

Strategy: expert-parallel. Experts (contexts) are sharded 8-per-core.
The host routes samples to the core that owns their expert (this is the
all-to-all, done during input sharding), padding each expert's sample
group to a fixed capacity CAP so the SPMD device program is fully
static. Each core then does 8 dense (CAP x 256) @ (256 x 256) matmuls —
weights are read from HBM exactly once across the whole device, which is
what the memory-bound roofline wants. The host scatters the device
output back to the original sample order.

v2 changes vs the f32 baseline (which was fp32-PE-bound: fp32 streams at
4 cycles/row on the PE, ~13.7us/iter at the 1.2GHz mid p-state):
  - bf16 everywhere on device (x, W, out). Max rel err on the problem
    data is 3.0e-3 against the f32 reference (gate is 2e-2). PE streams
    at 1 cycle/row (4x), DMA bytes halve.
  - optional fp8 e3m4 weights (W_SCALE pre-scaling): halves W DMA again;
    rel err 1.2e-2.
  - CAP=48 (max per-expert count for this data is 45; was 64), cutting
    the padded x/out traffic 25%.
  - input DMAs issued from gpsimd + scalar in parallel (the f32 baseline
    serialized 9 dma_starts on sync at ~565ns each); output DMAs from
    sync. Expert pairs sit at PSUM partition offsets 0/64 (PE tile
    positions allow only {0,64} for <=64-row tiles).
  - optional PE warmup matmuls per iteration (release the 1.2->2.4GHz
    p-state ramp while input DMAs stream).

Device program per core (raw Bass, manual semaphores):
  gpsimd : DMA xt slices + first half of W groups into SBUF
  scalar : DMA second half of W groups
  tensor : optional warmup matmuls; per expert, 2 accumulating matmuls
           (K=256 split in 2) into a PSUM half-bank at offset 0/64
  vector : PSUM -> SBUF copy per expert pair, f32 -> bf16
  sync   : DMA each pair's (128, 256) bf16 result back to DRAM

niter > 1 replicates the body with double-buffered inputs and WAR
semaphore chaining — used by the benchmark harness to measure
steady-state per-iteration HW time via wall-clock slope. serial=True
chains iterations through the output-DMA completion semaphores so each
iteration is a faithful isolated cold call (no cross-iteration overlap;
warmup matmuls are gated the same way and their cost is included).
"""

import numpy as np
from contextlib import ExitStack

import ml_dtypes

B, D, U, C = 2048, 256, 256, 64
NCORES = 8
EPC = C // NCORES  # experts per core
CAP = 48           # per-expert sample capacity (padded); data max is 45

WDT = "bf16"       # device weight dtype: "bf16" | "fp8e3"
W_SCALE = 256.0    # fp8 weights are pre-scaled by this; host divides out

_prog_cache: dict = {}


def _build_program(cap: int, niter: int = 1, serial: bool = False,
                   wdt: str = WDT, wgroup: int = 2, xsplit: int = 2,
                   warmup: int = 0, out_engine: str = "sync"):
    import concourse.bass as bass
    from concourse import mybir
    from concourse.bass import compact_to_ranges

    f32 = mybir.dt.float32
    bf16 = mybir.dt.bfloat16
    wdtype = {"bf16": bf16, "fp8e3": mybir.dt.float8e3,
              "fp8e4": mybir.dt.float8e4}[wdt]
    assert cap % 16 == 0 and 16 <= cap <= 64
    assert EPC % wgroup == 0 and EPC % xsplit == 0
    npair = EPC // 2
    ngrp = EPC // wgroup
    eps = EPC // xsplit  # experts per xt slice
    nc = bass.Bass()
    # xt: x^T with samples grouped by expert, [D, EPC*cap]
    xt = nc.declare_dram_parameter("xt", [D, EPC * cap], bf16, isOutput=False)
    w = nc.declare_dram_parameter("w", [EPC, D, U], wdtype, isOutput=False)
    # per pair: expert 2p at rows 0:cap, expert 2p+1 at rows 64:64+cap
    out = nc.declare_dram_parameter("out", [npair, 128, U], bf16, isOutput=True)

    NSET = 2 if niter > 1 else 1

    with ExitStack() as ctx:
        # xt SBUF: [128, 2, EPC*cap] — the two K-chunks in a free dim
        sb_xt = [
            ctx.enter_context(nc.sbuf_tensor(f"sb_xt{s}", [128, 2, EPC * cap], bf16))
            for s in range(NSET)
        ]
        sb_w = [
            [
                ctx.enter_context(
                    nc.sbuf_tensor(f"sb_w{g}_{s}", [128, wgroup, 2, U], wdtype)
                )
                for s in range(NSET)
            ]
            for g in range(ngrp)
        ]
        sb_out = [
            ctx.enter_context(nc.sbuf_tensor(f"sb_out{p}", [128, U], bf16))
            for p in range(npair)
        ]
        # one full PSUM bank per expert pair
        ps = [
            ctx.enter_context(nc.psum_tensor(f"ps{p}", [128, 512], f32))
            for p in range(npair)
        ]
        if warmup:
            sb_warm = ctx.enter_context(nc.sbuf_tensor("sb_warm", [128, 512], bf16))
            ps_warm = ctx.enter_context(nc.psum_tensor("ps_warm", [128, 512], f32))

        # Dedicated sems per buffer group: a wait threshold on a sem that
        # counts several in-flight DMAs is unsound (a DMA's +16 completion
        # is split +1 across 16 SDMA engines, so a later DMA's increments
        # can satisfy an earlier DMA's threshold while it still has a
        # straggler engine). One sem per buffer makes thresholds exact.
        warm_sem = ctx.enter_context(nc.semaphore("warm_sem"))
        ps_init_sem = ctx.enter_context(nc.semaphore("ps_init_sem"))
        xt_sem = [ctx.enter_context(nc.semaphore(f"xt_sem{t}")) for t in range(xsplit)]
        w_sem = [ctx.enter_context(nc.semaphore(f"w_sem{g}")) for g in range(ngrp)]
        mm_sem = ctx.enter_context(nc.semaphore("mm_sem"))
        cp_sem = ctx.enter_context(nc.semaphore("cp_sem"))
        out_sem = [ctx.enter_context(nc.semaphore(f"out_sem{p}")) for p in range(npair)]

        # Semaphores are NOT cleared when a loaded NEFF is re-executed, so
        # absolute wait thresholds would be stale on the second run. Clear
        # the whole kernel sem range up front (same preamble the BIR
        # lowering path emits), then a pseudo-sync barrier keeps every
        # engine parked until the clears retire.
        for sem_range in compact_to_ranges(
            [s for s in nc._kernel_sem_range if s not in nc.barrier_sems]
        ):
            nc.gpsimd.dma_reset(sem_range)
            nc.gpsimd.sem_clear(sem_range)
        nc._nrt_pseudo_barrier()
        if warmup:
            nc.gpsimd.memset(sb_warm[:, :], 0.0)
            nc.gpsimd.sem_inc(warm_sem, 1)
        # One-time zero of the PSUM pair banks: rows outside the expert
        # capacity (cap:64, 64+cap:128) are never written by matmuls but ARE
        # copied/DMAed (full-128 ops are faster than garbage-skipping APs);
        # the host ignores them. Matmuls only rewrite their own rows, so a
        # single preamble memset keeps the pad rows finite forever.
        for p in range(npair):
            nc.vector.memset(ps[p][:, :], 0.0)
        nc.vector.sem_inc(ps_init_sem, 1)

        block = ctx.enter_context(nc.Block())

        # DRAM access patterns with both K-chunks in the free dim
        xt_r = xt.rearrange("(k p) c -> p k c", k=2)        # [128, 2, EPC*cap]
        w_r = [
            w[g * wgroup:(g + 1) * wgroup].rearrange("e (k p) u -> p e k u", k=2)
            for g in range(ngrp)
        ]

        g_lo = range(0, (ngrp + 1) // 2)      # gpsimd-issued W groups
        g_hi = range((ngrp + 1) // 2, ngrp)   # scalar-issued W groups

        def serial_gate(eng, i):
            if serial and i >= 1:
                for p in range(npair):
                    eng.wait_ge(out_sem[p], 16 * i)

        @block.gpsimd
        def _(gpsimd):
            for i in range(niter):
                s = i % NSET
                serial_gate(gpsimd, i)
                for t in range(xsplit):
                    if i >= 2:
                        # xt set s was read by all matmuls of iter i-2
                        gpsimd.wait_ge(mm_sem, 8 * (i - 1))
                    gpsimd.dma_start(
                        sb_xt[s][:, :, t * eps * cap:(t + 1) * eps * cap],
                        xt_r[:, :, t * eps * cap:(t + 1) * eps * cap],
                    ).then_inc(xt_sem[t], 16)
                for g in g_lo:
                    if i >= 2:
                        gpsimd.wait_ge(mm_sem, 8 * (i - 2) + (g + 1) * wgroup)
                    gpsimd.dma_start(sb_w[g][s][:, :, :, :], w_r[g]).then_inc(
                        w_sem[g], 16)

        @block.scalar
        def _(scalar):
            for i in range(niter):
                s = i % NSET
                serial_gate(scalar, i)
                for g in g_hi:
                    if i >= 2:
                        scalar.wait_ge(mm_sem, 8 * (i - 2) + (g + 1) * wgroup)
                    scalar.dma_start(sb_w[g][s][:, :, :, :], w_r[g]).then_inc(
                        w_sem[g], 16)

        @block.tensor
        def _(tensor):
            if warmup:
                tensor.wait_ge(warm_sem, 1)
            tensor.wait_ge(ps_init_sem, 1)
            for i in range(niter):
                serial_gate(tensor, i)
                if warmup:
                    # Dummy matmuls: sustained PE activity walks the p-state
                    # up (0.65 -> 1.2 -> 2.4 GHz) while input DMAs stream, so
                    # the real matmuls run faster even in a cold call. Gated
                    # by the serial chain above so each serial iteration pays
                    # for its own ramp, like a real cold call would.
                    for _ in range(warmup):
                        tensor.matmul(
                            ps_warm[:, :], sb_warm[:, 0:128], sb_warm[:, :],
                            start=True, stop=True,
                        )
                s = i % NSET
                for j in range(EPC):
                    p, half = j // 2, j % 2
                    g, e_local = j // wgroup, j % wgroup
                    t = j // eps
                    if j % eps == 0:
                        tensor.wait_ge(xt_sem[t], 16 * (i + 1))
                    if e_local == 0:
                        tensor.wait_ge(w_sem[g], 16 * (i + 1))
                    if i >= 1 and half == 0:
                        # pair bank p was copied out during iter i-1
                        tensor.wait_ge(cp_sem, npair * (i - 1) + p + 1)
                    for k in range(2):
                        mm = tensor.matmul(
                            ps[p][half * 64:half * 64 + cap, 0:U],
                            sb_xt[s][:, k, j * cap:(j + 1) * cap],
                            sb_w[g][s][:, e_local, k, :],
                            start=(k == 0),
                            stop=(k == 1),
                        )
                    mm.then_inc(mm_sem, 1)

        @block.vector
        def _(vector):
            for i in range(niter):
                for p in range(npair):
                    vector.wait_ge(mm_sem, 8 * i + 2 * p + 2)
                    if i >= 1:
                        vector.wait_ge(out_sem[p], 16 * i)
                    vector.tensor_copy(
                        sb_out[p][:, :], ps[p][:, 0:U]
                    ).then_inc(cp_sem, 1)

        @block.sync
        def _(sync):
            for i in range(niter):
                for p in range(npair):
                    sync.wait_ge(cp_sem, npair * i + p + 1)
                    sync.dma_start(out[p], sb_out[p][:, :]).then_inc(out_sem[p], 16)
            for p in range(npair):
                sync.wait_ge(out_sem[p], 16 * niter)

    return nc


def _route(content_idx: np.ndarray, x: np.ndarray, cap: int):
    """Sort samples by expert; compute per-core padded x^T shards (bf16)."""
    idx = content_idx.reshape(-1).astype(np.int64)
    order = np.argsort(idx, kind="stable")
    e_sorted = idx[order]
    counts = np.bincount(idx, minlength=C)
    while counts.max() > cap:
        cap += 16
    start = np.zeros(C, dtype=np.int64)
    start[1:] = np.cumsum(counts)[:-1]
    slot = np.arange(B) - start[e_sorted]
    core = e_sorted // EPC
    col = (e_sorted % EPC) * cap + slot

    xt_all = np.zeros((NCORES, D, EPC * cap), dtype=np.float32)
    xt_all[core, :, col] = x[order]
    return cap, order, core, col, xt_all


def _unshard(outs: np.ndarray, order, core, col, cap: int) -> np.ndarray:
    """Scatter per-core padded device output back to original sample order.

    outs: (NCORES, npair, 128, U) bf16; expert pair p holds local expert 2p
    at rows 0:cap and 2p+1 at rows 64:64+cap.
    """
    scale = W_SCALE if WDT.startswith("fp8") else 1.0
    out_full = np.empty((B, U), dtype=np.float32)
    jl = col // cap          # local expert index
    slot = col % cap
    out_full[order] = outs[core, jl // 2, (jl % 2) * 64 + slot, :].astype(np.float32)
    if scale != 1.0:
        out_full /= scale
    return out_full


def _make_in_maps(xt_all: np.ndarray, kernel_w: np.ndarray):
    bf16 = ml_dtypes.bfloat16
    if WDT == "bf16":
        wdev = kernel_w.reshape(NCORES, EPC, D, U).astype(bf16)
    elif WDT == "fp8e3":
        wdev = (kernel_w.reshape(NCORES, EPC, D, U) * W_SCALE).astype(
            ml_dtypes.float8_e3m4)
    elif WDT == "fp8e4":
        wdev = (kernel_w.reshape(NCORES, EPC, D, U) * W_SCALE).astype(
            ml_dtypes.float8_e4m3)
    else:
        raise ValueError(WDT)
    wdev = np.ascontiguousarray(wdev)
    xt16 = xt_all.astype(bf16)
    return [{"xt": xt16[c], "w": wdev[c]} for c in range(NCORES)]


def kernel(content_idx: np.ndarray, x: np.ndarray, kernel: np.ndarray) -> np.ndarray:
    from concourse.bass_utils import run_bass_kernel_spmd

    cap, order, core, col, xt_all = _route(content_idx, x, CAP)
    if cap > 64:
        # Pathologically skewed routing (an expert holds >64 samples) can't
        # use the static pair-packed program (PE tile offsets allow only
        # {0,64}). Unreachable for the fixed-seed problem data; fall back to
        # a host computation to stay correct.
        idx = content_idx.reshape(-1).astype(np.int64)
        return np.einsum("bd,bdu->bu", x.astype(np.float32),
                         kernel.astype(np.float32)[idx]).astype(np.float32)

    key = (cap, 1)
    if key not in _prog_cache:
        _prog_cache[key] = _build_program(cap, 1)
    nc = _prog_cache[key]

    in_maps = _make_in_maps(xt_all, kernel)
    res = run_bass_kernel_spmd(nc, in_maps, list(range(NCORES)))
    outs = np.stack([np.asarray(res.results[c]["out"]) for c in range(NCORES)])
    return _unshard(outs, order, core, col, cap)


# revision 17
# speedup vs baseline: 1.4598x; 1.2440x over previous
"""MoE routed matmul on 8 NeuronCores (Trainium2, Bass).

Problem: out[b] = x[b] @ W[idx[b]]  with  x:(2048,256), W:(64,256,256),
idx:(2048,1) int32.

Strategy: expert-parallel. Experts (contexts) are sharded 8-per-core.
The host routes samples to the core that owns their expert (this is the
all-to-all, done during input sharding), padding each expert's sample
group to a fixed capacity CAP so the SPMD device program is fully
static. Each core then does 8 dense (CAP x 256) @ (256 x 256) matmuls —
weights are read from HBM exactly once across the whole device, which is
what the memory-bound roofline wants. The host scatters the device
output back to the original sample order.

Performance structure (vs the 15.6us f32 baseline, which was PE-bound:
fp32 streams at 4 cycles/row at the 1.2GHz mid p-state):
  - x and the output travel as bf16, weights as fp8 e3m4 pre-scaled by
    W_SCALE (rel err 1.2e-2 on the problem data vs the 2e-2 gate;
    bf16-everywhere is 3.0e-3 and one flag away). PE streams the moving
    operand at 1 cycle/row for both.
  - ALL device inputs are packed on the host into one partition-major
    byte image [128, NB] (xt bf16, then each expert's W). The device
    DMAs it in a few large fully-contiguous column chunks — DMA configs
    (~650ns) and the shared HWDGE descriptor-gen unit (~630ns/DMA) are
    the serial bottleneck, not bytes, so fewer+bigger beats many+small.
    Matmul operands are bitcast views into the image.
  - DMA issue only on SP/Activation (HWDGE). gpsimd's software DGE costs
    ~1.1us/DMA on the Q7 cores; DVE can't issue DMAs at all.
  - CAP=48 (max per-expert count for this data is 45): expert pairs sit
    in one PSUM bank at partition offsets 0/64 (PE tile positions allow
    only {0,64}), copied out full-128-wide; the host skips the pad rows.
  - warmup matmuls on zeroed SBUF bridge the PE p-state ramp
    (0.65 -> 1.2 -> 2.4 GHz after 3us continuously busy) across the
    input-DMA head so the real matmuls run at full clock.

Device program per core (raw Bass, manual semaphores):
  sync   : DMA input-image chunk 0 (xt + first experts), last out chunk
  scalar : DMA remaining input chunks, first out chunk(s)
  tensor : warmup matmuls; per expert, 2 accumulating matmuls (K=256
           split in 2) into a PSUM half-bank at offset 0/64
  vector : PSUM -> SBUF copy per expert pair, f32 -> bf16

niter > 1 replicates the body with double-buffered inputs and WAR
semaphore chaining — used by the benchmark harness to measure
steady-state per-iteration HW time via wall-clock slope. serial=True
chains every engine's iteration i behind iteration i-1's output-DMA
completion semaphores, so each iteration is a faithful isolated cold
call (no cross-iteration overlap; warmup matmuls are gated the same way
and their cost is included).
"""

import numpy as np
from contextlib import ExitStack

import ml_dtypes

B, D, U, C = 2048, 256, 256, 64
NCORES = 8
EPC = C // NCORES  # experts per core
CAP = 48           # per-expert sample capacity (padded); data max is 45

WDT = "fp8e3"      # device weight dtype: "bf16" | "fp8e3"
W_SCALE = 256.0    # fp8 weights are pre-scaled by this; host divides out

# input-image chunk split: experts per input DMA (chunk 0 also carries xt)
INSPLIT = (1, 3, 4)
OUT_CHUNKS = 2
WARMUP = 6

_prog_cache: dict = {}


def _wsize(wdt: str) -> int:
    return 2 if wdt == "bf16" else 1


def _layout(cap: int, wdt: str):
    """Byte layout of the packed input image (per partition).

    Per expert j (interleaved so any expert range is byte-contiguous):
      [xt_k0 (cap bf16) | xt_k1 (cap bf16) | w_k0 (U wdt) | w_k1 (U wdt)]
    """
    xeb = 2 * cap * 2               # xt bytes per expert (both K-chunks)
    wb = 2 * U * _wsize(wdt)        # W bytes per expert (both K-chunks)
    eb = xeb + wb
    nb = EPC * eb
    return xeb, wb, eb, nb


def _build_program(cap: int, niter: int = 1, serial: bool = False,
                   wdt: str = WDT, insplit=INSPLIT, out_chunks: int = OUT_CHUNKS,
                   warmup: int = WARMUP, trig_out: bool = False):
    import concourse.bass as bass
    from concourse import mybir
    from concourse.bass import compact_to_ranges

    f32 = mybir.dt.float32
    bf16 = mybir.dt.bfloat16
    u8 = mybir.dt.uint8
    i32 = mybir.dt.int32
    wdtype = {"bf16": bf16, "fp8e3": mybir.dt.float8e3,
              "fp8e4": mybir.dt.float8e4}[wdt]
    assert cap % 16 == 0 and 16 <= cap <= 64
    assert sum(insplit) == EPC
    npair = EPC // 2
    assert npair % out_chunks == 0
    ppc = npair // out_chunks  # pairs per output chunk
    xeb, wb, eb, nb = _layout(cap, wdt)

    # input chunk column ranges [a, b) in the byte image + expert coverage
    chunks = []
    e0 = 0
    for ne in insplit:
        chunks.append((e0 * eb, (e0 + ne) * eb, e0, e0 + ne))
        e0 += ne
    echunk = {}
    for ci, (_, _, ea, ebnd) in enumerate(chunks):
        for j in range(ea, ebnd):
            echunk[j] = ci
    nchunk = len(chunks)

    nc = bass.Bass()
    inp = nc.declare_dram_parameter("inp", [128, nb], u8, isOutput=False)
    # per pair: expert 2p at rows 0:cap, expert 2p+1 at rows 64:64+cap
    out = nc.declare_dram_parameter("out", [npair, 128, U], bf16, isOutput=True)

    NSET = 2 if niter > 1 else 1

    with ExitStack() as ctx:
        sb_in = [
            ctx.enter_context(nc.sbuf_tensor(f"sb_in{s}", [128, nb], u8))
            for s in range(NSET)
        ]
        # one contiguous out staging tensor so a chunk of pairs goes out in
        # one DMA: pair p lives at columns [p*U, (p+1)*U)
        sb_out = ctx.enter_context(nc.sbuf_tensor("sb_out", [128, npair * U], bf16))
        # one full PSUM bank per expert pair
        ps = [
            ctx.enter_context(nc.psum_tensor(f"ps{p}", [128, 512], f32))
            for p in range(npair)
        ]
        if warmup:
            sb_warm = ctx.enter_context(nc.sbuf_tensor("sb_warm", [128, 512], bf16))
            ps_warm = ctx.enter_context(nc.psum_tensor("ps_warm", [128, 512], f32))
        if trig_out:
            # zero ctx indices for the kv_writeback-shaped output DMA
            sb_idx = ctx.enter_context(nc.sbuf_tensor("sb_idx", [128, npair], i32))

        # Dedicated sems per buffer group: a wait threshold on a sem that
        # counts several in-flight DMAs is unsound (a DMA's +16 completion
        # is split +1 across 16 SDMA engines, so a later DMA's increments
        # can satisfy an earlier DMA's threshold while it still has a
        # straggler engine). One sem per buffer makes thresholds exact.
        warm_sem = ctx.enter_context(nc.semaphore("warm_sem"))
        ps_init_sem = ctx.enter_context(nc.semaphore("ps_init_sem"))
        in_sem = [ctx.enter_context(nc.semaphore(f"in_sem{t}"))
                  for t in range(nchunk)]
        mm_sem = ctx.enter_context(nc.semaphore("mm_sem"))
        cp_sem = ctx.enter_context(nc.semaphore("cp_sem"))
        out_sem = [ctx.enter_context(nc.semaphore(f"out_sem{c}"))
                   for c in range(out_chunks)]
        if trig_out:
            prep_sem = ctx.enter_context(nc.semaphore("prep_sem"))

        # Semaphores are NOT cleared when a loaded NEFF is re-executed, so
        # absolute wait thresholds would be stale on the second run. Clear
        # the whole kernel sem range up front (same preamble the BIR
        # lowering path emits), then a pseudo-sync barrier keeps every
        # engine parked until the clears retire.
        for sem_range in compact_to_ranges(
            [s for s in nc._kernel_sem_range if s not in nc.barrier_sems]
        ):
            nc.gpsimd.dma_reset(sem_range)
            nc.gpsimd.sem_clear(sem_range)
        nc._nrt_pseudo_barrier()
        if warmup:
            nc.gpsimd.memset(sb_warm[:, :], 0.0)
            nc.gpsimd.sem_inc(warm_sem, 1)
        if trig_out:
            from concourse import library_config
            nc.gpsimd.load_library(library_config.attn)
            nc.gpsimd.memset(sb_idx[:, :], 0)
        # One-time zero of the PSUM pair banks: rows outside the expert
        # capacity (cap:64, 64+cap:128) are never written by matmuls but ARE
        # copied/DMAed (full-128 ops beat garbage-skipping APs); the host
        # ignores them. Matmuls only rewrite their own rows, so a single
        # preamble memset keeps the pad rows finite forever.
        for p in range(npair):
            nc.vector.memset(ps[p][:, :], 0.0)
        nc.vector.sem_inc(ps_init_sem, 1)

        block = ctx.enter_context(nc.Block())

        def xt_ap(s, j, k):
            a = j * eb + k * cap * 2
            return sb_in[s][:, a:a + cap * 2].bitcast(bf16)

        def w_ap(s, j, k):
            a = j * eb + xeb + k * (wb // 2)
            return sb_in[s][:, a:a + wb // 2].bitcast(wdtype)

        # out chunk c: DRAM [ppc, 128, U] <- SBUF [128, ppc, U]
        out_r = [
            out[c * ppc:(c + 1) * ppc].rearrange("p r u -> r p u")
            for c in range(out_chunks)
        ]

        def serial_gate(eng, i):
            if serial and i >= 1:
                for c in range(out_chunks):
                    eng.wait_ge(out_sem[c], 16 * i)

        def issue_in(eng, i, ci):
            s = i % NSET
            a, b, ea, ebnd = chunks[ci]
            if i >= 2:
                # chunk ci of set s was read by its own experts' matmuls of
                # iter i-2 (the chunk carries those experts' xt AND W)
                eng.wait_ge(mm_sem, 8 * (i - 2) + ebnd)
            eng.dma_start(sb_in[s][:, a:b], inp[:, a:b]).then_inc(in_sem[ci], 16)

        def issue_out(eng, i, c):
            eng.wait_ge(cp_sem, npair * i + ppc * (c + 1))
            eng.dma_start(
                out_r[c],
                sb_out[:, c * ppc * U:(c + 1) * ppc * U].rearrange(
                    "r (p u) -> r p u", p=ppc),
            ).then_inc(out_sem[c], 16)

        @block.sync
        def _(sync):
            for i in range(niter):
                serial_gate(sync, i)
                issue_in(sync, i, 0)
                if not trig_out:
                    issue_out(sync, i, out_chunks - 1)
            for c in range(out_chunks):
                sync.wait_ge(out_sem[c], 16 * niter)

        @block.scalar
        def _(scalar):
            for i in range(niter):
                serial_gate(scalar, i)
                for ci in range(1, nchunk):
                    issue_in(scalar, i, ci)
                if not trig_out:
                    for c in range(out_chunks - 1):
                        issue_out(scalar, i, c)

        if trig_out:
            # Output DMAs via gpsimd's SWDGE prepare/trigger split: the
            # ~1us/DMA Q7 descriptor generation happens during the input-DMA
            # head (Pool is otherwise idle), so after the last PSUM copy only
            # the cheap ring-doorbell write + transfer + completion sem remain
            # on the critical path (vs ~1.9us of config+HWDGE+DGE-delay for a
            # plain dma_start). kv_writeback with all-zero ctx indices is a
            # plain transposing SBUF->DRAM write.
            @block.gpsimd
            def _(gpsimd):
                for i in range(niter):
                    serial_gate(gpsimd, i)
                    for c in range(out_chunks):
                        gpsimd.kv_writeback(
                            out[c * ppc:(c + 1) * ppc].rearrange(
                                "p (r o) u -> p r o u", o=1),
                            sb_out[:, c * ppc * U:(c + 1) * ppc * U].rearrange(
                                "r (o p u) -> r o p u", o=1, p=ppc),
                            sb_idx[:, c * ppc:(c + 1) * ppc],
                            prepare_only=True,
                            sem=out_sem[c],
                        ).then_inc(prep_sem, 1)
                    for c in range(out_chunks):
                        gpsimd.wait_ge(prep_sem, out_chunks * i + c + 1)
                        gpsimd.wait_ge(cp_sem, npair * i + ppc * (c + 1))
                        gpsimd.trigger_dma(count=1)

        @block.tensor
        def _(tensor):
            if warmup:
                tensor.wait_ge(warm_sem, 1)
            tensor.wait_ge(ps_init_sem, 1)
            for i in range(niter):
                serial_gate(tensor, i)
                if warmup:
                    # Dummy matmuls: sustained PE activity walks the p-state
                    # up (0.65 -> 1.2 -> 2.4 GHz) while input DMAs stream, so
                    # the real matmuls run at full clock even in a cold call.
                    # Gated by the serial chain above so each serial iteration
                    # pays for its own ramp, like a real cold call would.
                    for _ in range(warmup):
                        tensor.matmul(
                            ps_warm[:, :], sb_warm[:, 0:128], sb_warm[:, :],
                            start=True, stop=True,
                        )
                s = i % NSET
                for j in range(EPC):
                    p, half = j // 2, j % 2
                    if j == 0 or echunk[j] != echunk[j - 1]:
                        tensor.wait_ge(in_sem[echunk[j]], 16 * (i + 1))
                    if i >= 1 and half == 0:
                        # pair bank p was copied out during iter i-1
                        tensor.wait_ge(cp_sem, npair * (i - 1) + p + 1)
                    for k in range(2):
                        mm = tensor.matmul(
                            ps[p][half * 64:half * 64 + cap, 0:U],
                            xt_ap(s, j, k),
                            w_ap(s, j, k),
                            start=(k == 0),
                            stop=(k == 1),
                        )
                    mm.then_inc(mm_sem, 1)

        @block.vector
        def _(vector):
            for i in range(niter):
                for p in range(npair):
                    vector.wait_ge(mm_sem, 8 * i + 2 * p + 2)
                    if i >= 1:
                        # sb_out chunk was DMAed out during iter i-1
                        vector.wait_ge(out_sem[p // ppc], 16 * i)
                    vector.tensor_copy(
                        sb_out[:, p * U:(p + 1) * U], ps[p][:, 0:U]
                    ).then_inc(cp_sem, 1)

    return nc


def _route(content_idx: np.ndarray, x: np.ndarray, cap: int):
    """Sort samples by expert; compute per-core padded x^T shards."""
    idx = content_idx.reshape(-1).astype(np.int64)
    order = np.argsort(idx, kind="stable")
    e_sorted = idx[order]
    counts = np.bincount(idx, minlength=C)
    while counts.max() > cap:
        cap += 16
    start = np.zeros(C, dtype=np.int64)
    start[1:] = np.cumsum(counts)[:-1]
    slot = np.arange(B) - start[e_sorted]
    core = e_sorted // EPC
    col = (e_sorted % EPC) * cap + slot

    xt_all = np.zeros((NCORES, D, EPC * cap), dtype=np.float32)
    xt_all[core, :, col] = x[order]
    return cap, order, core, col, xt_all


def _unshard(outs: np.ndarray, order, core, col, cap: int) -> np.ndarray:
    """Scatter per-core padded device output back to original sample order.

    outs: (NCORES, npair, 128, U) bf16; expert pair p holds local expert 2p
    at rows 0:cap and 2p+1 at rows 64:64+cap.
    """
    scale = W_SCALE if WDT.startswith("fp8") else 1.0
    out_full = np.empty((B, U), dtype=np.float32)
    jl = col // cap          # local expert index
    slot = col % cap
    out_full[order] = outs[core, jl // 2, (jl % 2) * 64 + slot, :].astype(np.float32)
    if scale != 1.0:
        out_full /= scale
    return out_full


def _make_in_maps(xt_all: np.ndarray, kernel_w: np.ndarray):
    """Build the packed per-core input byte image [128, NB]."""
    bf16 = ml_dtypes.bfloat16
    cap = xt_all.shape[2] // EPC
    xeb, wb, eb, nb = _layout(cap, WDT)
    if WDT == "bf16":
        wdev = kernel_w.reshape(NCORES, EPC, D, U).astype(bf16)
    elif WDT == "fp8e3":
        wdev = (kernel_w.reshape(NCORES, EPC, D, U) * W_SCALE).astype(
            ml_dtypes.float8_e3m4)
    elif WDT == "fp8e4":
        wdev = (kernel_w.reshape(NCORES, EPC, D, U) * W_SCALE).astype(
            ml_dtypes.float8_e4m3)
    else:
        raise ValueError(WDT)

    # per expert j: [xt_k0 | xt_k1 | w_k0 | w_k1], all indexed by partition p
    img = np.empty((NCORES, 128, EPC, eb), dtype=np.uint8)
    xt16 = xt_all.astype(bf16)                       # [NC, 256, EPC*cap]
    # [c, k, p, e, cap] -> [c, p, e, k, cap]
    xtb = xt16.reshape(NCORES, 2, 128, EPC, cap).transpose(0, 2, 3, 1, 4)
    img[:, :, :, :xeb] = np.ascontiguousarray(xtb).view(np.uint8).reshape(
        NCORES, 128, EPC, xeb)
    # [c, e, k, p, u] -> [c, p, e, k, u]
    wkb = wdev.reshape(NCORES, EPC, 2, 128, U).transpose(0, 3, 1, 2, 4)
    img[:, :, :, xeb:] = np.ascontiguousarray(wkb).view(np.uint8).reshape(
        NCORES, 128, EPC, wb)
    img = img.reshape(NCORES, 128, nb)
    return [{"inp": img[c]} for c in range(NCORES)]


def kernel(content_idx: np.ndarray, x: np.ndarray, kernel: np.ndarray) -> np.ndarray:
    from concourse.bass_utils import run_bass_kernel_spmd

    cap, order, core, col, xt_all = _route(content_idx, x, CAP)
    if cap > 64:
        # Pathologically skewed routing (an expert holds >64 samples) can't
        # use the static pair-packed program (PE tile offsets allow only
        # {0,64}). Unreachable for the fixed-seed problem data; fall back to
        # a host computation to stay correct.
        idx = content_idx.reshape(-1).astype(np.int64)
        return np.einsum("bd,bdu->bu", x.astype(np.float32),
                         kernel.astype(np.float32)[idx]).astype(np.float32)

    key = (cap, 1)
    if key not in _prog_cache:
        _prog_cache[key] = _build_program(cap, 1)
    nc = _prog_cache[key]

    in_maps = _make_in_maps(xt_all, kernel)
    res = run_bass_kernel_spmd(nc, in_maps, list(range(NCORES)))
    outs = np.stack([np.asarray(res.results[c]["out"]) for c in range(NCORES)])
    return _unshard(outs, order, core, col, cap)


# revision 20
# speedup vs baseline: 1.9816x; 1.3575x over previous
"""MoE routed matmul on 8 NeuronCores (Trainium2, Bass).

Problem: out[b] = x[b] @ W[idx[b]]  with  x:(2048,256), W:(64,256,256),
idx:(2048,1) int32.

Strategy: expert-parallel. Experts (contexts) are sharded 8-per-core.
The host routes samples to the core that owns their expert (this is the
all-to-all, done during input sharding), padding each expert's sample
group to a fixed capacity CAP so the SPMD device program is fully
static. Each core then does 8 dense (CAP x 256) @ (256 x 256) matmuls —
weights are read from HBM exactly once across the whole device, which is
what the memory-bound roofline wants. The host scatters the device
output back to the original sample order.

Performance structure (vs the 15.6us f32 baseline, which was PE-bound:
fp32 streams at 4 cycles/row at the 1.2GHz mid p-state):
  - x and the output travel as bf16, weights as fp8 e3m4 pre-scaled by
    W_SCALE (rel err 1.2e-2 on the problem data vs the 2e-2 gate;
    bf16-everywhere is 3.0e-3 and one flag away). PE streams the moving
    operand at 1 cycle/row for both.
  - ALL device inputs are packed on the host into one partition-major
    byte image [128, NB] (xt bf16, then each expert's W). The device
    DMAs it in a few large fully-contiguous column chunks — DMA configs
    (~650ns) and the shared HWDGE descriptor-gen unit (~630ns/DMA) are
    the serial bottleneck, not bytes, so fewer+bigger beats many+small.
    Matmul operands are bitcast views into the image.
  - DMA issue only on SP/Activation (HWDGE). gpsimd's software DGE costs
    ~1.1us/DMA on the Q7 cores; DVE can't issue DMAs at all.
  - CAP=48 (max per-expert count for this data is 45): expert pairs sit
    in one PSUM bank at partition offsets 0/64 (PE tile positions allow
    only {0,64}), copied out full-128-wide; the host skips the pad rows.
  - warmup matmuls on zeroed SBUF bridge the PE p-state ramp
    (0.65 -> 1.2 -> 2.4 GHz after 3us continuously busy) across the
    input-DMA head so the real matmuls run at full clock.

Device program per core (raw Bass, manual semaphores):
  sync   : DMA input-image chunk 0 (xt + first experts), last out chunk
  scalar : DMA remaining input chunks, first out chunk(s)
  tensor : warmup matmuls; per expert, 2 accumulating matmuls (K=256
           split in 2) into a PSUM half-bank at offset 0/64
  vector : PSUM -> SBUF copy per expert pair, f32 -> bf16

niter > 1 replicates the body with double-buffered inputs and WAR
semaphore chaining — used by the benchmark harness to measure
steady-state per-iteration HW time via wall-clock slope. serial=True
chains every engine's iteration i behind iteration i-1's output-DMA
completion semaphores, so each iteration is a faithful isolated cold
call (no cross-iteration overlap; warmup matmuls are gated the same way
and their cost is included).
"""

import numpy as np
from contextlib import ExitStack

import ml_dtypes

B, D, U, C = 2048, 256, 256, 64
NCORES = 8
EPC = C // NCORES  # experts per core
CAP = 48           # per-expert sample capacity (padded); data max is 45

WDT = "fp8e3"      # device weight dtype: "bf16" | "fp8e3"
W_SCALE = 256.0    # fp8 weights are pre-scaled by this; host divides out

# input-image chunk split: experts per input DMA (chunk 0 also carries xt)
INSPLIT = (1, 3, 4)
OUT_CHUNKS = 2
WARMUP = 6

_prog_cache: dict = {}


def _wsize(wdt: str) -> int:
    return 2 if wdt == "bf16" else 1


def _layout(cap: int, wdt: str):
    """Byte layout of the packed input image (per partition).

    Per expert j (interleaved so any expert range is byte-contiguous):
      [xt_k0 (cap bf16) | xt_k1 (cap bf16) | w_k0 (U wdt) | w_k1 (U wdt)]
    """
    xeb = 2 * cap * 2               # xt bytes per expert (both K-chunks)
    wb = 2 * U * _wsize(wdt)        # W bytes per expert (both K-chunks)
    eb = xeb + wb
    nb = EPC * eb
    return xeb, wb, eb, nb


def _build_program(cap: int, niter: int = 1, serial: bool = False,
                   wdt: str = WDT, insplit=INSPLIT, out_chunks: int = OUT_CHUNKS,
                   warmup: int = WARMUP, trig_out: bool = False,
                   pool_chunk: int = -1, tail_eng: str = "sp"):
    import concourse.bass as bass
    from concourse import mybir
    from concourse.bass import compact_to_ranges

    f32 = mybir.dt.float32
    bf16 = mybir.dt.bfloat16
    u8 = mybir.dt.uint8
    i32 = mybir.dt.int32
    wdtype = {"bf16": bf16, "fp8e3": mybir.dt.float8e3,
              "fp8e4": mybir.dt.float8e4}[wdt]
    assert cap % 16 == 0 and 16 <= cap <= 64
    assert sum(insplit) == EPC
    npair = EPC // 2
    osplit = ((npair // out_chunks,) * out_chunks
              if isinstance(out_chunks, int) else tuple(out_chunks))
    assert sum(osplit) == npair
    # pair range [oa, ob) per output chunk + chunk of each pair
    obnds, oa = [], 0
    for n in osplit:
        obnds.append((oa, oa + n))
        oa += n
    ochunk = {p: c for c, (a, b) in enumerate(obnds) for p in range(a, b)}
    out_chunks = len(osplit)
    xeb, wb, eb, nb = _layout(cap, wdt)

    # input chunk column ranges [a, b) in the byte image + expert coverage
    chunks = []
    e0 = 0
    for ne in insplit:
        chunks.append((e0 * eb, (e0 + ne) * eb, e0, e0 + ne))
        e0 += ne
    echunk = {}
    for ci, (_, _, ea, ebnd) in enumerate(chunks):
        for j in range(ea, ebnd):
            echunk[j] = ci
    nchunk = len(chunks)

    nc = bass.Bass()
    inp = nc.declare_dram_parameter("inp", [128, nb], u8, isOutput=False)
    # per pair: expert 2p at rows 0:cap, expert 2p+1 at rows 64:64+cap
    out = nc.declare_dram_parameter("out", [npair, 128, U], bf16, isOutput=True)

    NSET = 2 if niter > 1 else 1

    with ExitStack() as ctx:
        sb_in = [
            ctx.enter_context(nc.sbuf_tensor(f"sb_in{s}", [128, nb], u8))
            for s in range(NSET)
        ]
        # one contiguous out staging tensor so a chunk of pairs goes out in
        # one DMA: pair p lives at columns [p*U, (p+1)*U)
        sb_out = ctx.enter_context(nc.sbuf_tensor("sb_out", [128, npair * U], bf16))
        # one full PSUM bank per expert pair
        ps = [
            ctx.enter_context(nc.psum_tensor(f"ps{p}", [128, 512], f32))
            for p in range(npair)
        ]
        if warmup:
            sb_warm = ctx.enter_context(nc.sbuf_tensor("sb_warm", [128, 512], bf16))
            ps_warm = ctx.enter_context(nc.psum_tensor("ps_warm", [128, 512], f32))
        if trig_out:
            # zero ctx indices for the kv_writeback-shaped output DMA
            sb_idx = ctx.enter_context(nc.sbuf_tensor("sb_idx", [128, npair], i32))

        # Dedicated sems per buffer group: a wait threshold on a sem that
        # counts several in-flight DMAs is unsound (a DMA's +16 completion
        # is split +1 across 16 SDMA engines, so a later DMA's increments
        # can satisfy an earlier DMA's threshold while it still has a
        # straggler engine). One sem per buffer makes thresholds exact.
        warm_sem = ctx.enter_context(nc.semaphore("warm_sem"))
        ps_init_sem = ctx.enter_context(nc.semaphore("ps_init_sem"))
        in_sem = [ctx.enter_context(nc.semaphore(f"in_sem{t}"))
                  for t in range(nchunk)]
        mm_sem = ctx.enter_context(nc.semaphore("mm_sem"))
        cp_sem = ctx.enter_context(nc.semaphore("cp_sem"))
        out_sem = [ctx.enter_context(nc.semaphore(f"out_sem{c}"))
                   for c in range(out_chunks)]
        if trig_out:
            prep_sem = ctx.enter_context(nc.semaphore("prep_sem"))

        # Semaphores are NOT cleared when a loaded NEFF is re-executed, so
        # absolute wait thresholds would be stale on the second run. Clear
        # the whole kernel sem range up front (same preamble the BIR
        # lowering path emits), then a pseudo-sync barrier keeps every
        # engine parked until the clears retire.
        for sem_range in compact_to_ranges(
            [s for s in nc._kernel_sem_range if s not in nc.barrier_sems]
        ):
            nc.gpsimd.dma_reset(sem_range)
            nc.gpsimd.sem_clear(sem_range)
        nc._nrt_pseudo_barrier()
        if warmup:
            nc.gpsimd.memset(sb_warm[:, :], 0.0)
            nc.gpsimd.sem_inc(warm_sem, 1)
        if trig_out:
            from concourse import library_config
            nc.gpsimd.load_library(library_config.attn)
            nc.gpsimd.memset(sb_idx[:, :], 0)
        # One-time zero of the PSUM pair banks: rows outside the expert
        # capacity (cap:64, 64+cap:128) are never written by matmuls but ARE
        # copied/DMAed (full-128 ops beat garbage-skipping APs); the host
        # ignores them. Matmuls only rewrite their own rows, so a single
        # preamble memset keeps the pad rows finite forever.
        for p in range(npair):
            nc.vector.memset(ps[p][:, :], 0.0)
        nc.vector.sem_inc(ps_init_sem, 1)

        block = ctx.enter_context(nc.Block())

        def xt_ap(s, j, k):
            a = j * eb + k * cap * 2
            return sb_in[s][:, a:a + cap * 2].bitcast(bf16)

        def w_ap(s, j, k):
            a = j * eb + xeb + k * (wb // 2)
            return sb_in[s][:, a:a + wb // 2].bitcast(wdtype)

        # out chunk c: DRAM [nc_pairs, 128, U] <- SBUF [128, nc_pairs, U]
        out_r = [
            out[a:b].rearrange("p r u -> r p u")
            for a, b in obnds
        ]

        def serial_gate(eng, i):
            if serial and i >= 1:
                for c in range(out_chunks):
                    eng.wait_ge(out_sem[c], 16 * i)

        def issue_in(eng, i, ci):
            s = i % NSET
            a, b, ea, ebnd = chunks[ci]
            if i >= 2:
                # chunk ci of set s was read by its own experts' matmuls of
                # iter i-2 (the chunk carries those experts' xt AND W)
                eng.wait_ge(mm_sem, 8 * (i - 2) + ebnd)
            eng.dma_start(sb_in[s][:, a:b], inp[:, a:b]).then_inc(in_sem[ci], 16)

        def issue_out(eng, i, c):
            a, b = obnds[c]
            eng.wait_ge(cp_sem, npair * i + b)
            eng.dma_start(
                out_r[c],
                sb_out[:, a * U:b * U].rearrange("r (p u) -> r p u", p=b - a),
            ).then_inc(out_sem[c], 16)

        @block.sync
        def _(sync):
            for i in range(niter):
                serial_gate(sync, i)
                issue_in(sync, i, 0)
                if not trig_out and tail_eng == "sp":
                    issue_out(sync, i, out_chunks - 1)
            for c in range(out_chunks):
                sync.wait_ge(out_sem[c], 16 * niter)

        @block.scalar
        def _(scalar):
            for i in range(niter):
                serial_gate(scalar, i)
                for ci in range(1, nchunk):
                    if ci != pool_chunk:
                        issue_in(scalar, i, ci)
                if not trig_out:
                    for c in range(out_chunks - 1):
                        issue_out(scalar, i, c)

        if not trig_out and tail_eng == "pool":
            # The last out chunk via gpsimd software DGE: from the final
            # copy's semaphore, Pool's chain (~60ns dispatch + ~1.1us Q7
            # descriptor gen + transfer + sem) undercuts the HWDGE chain
            # (config 650 + gen 625 + DGE delay 650 + transfer + sem).
            @block.gpsimd
            def _(gpsimd):
                for i in range(niter):
                    serial_gate(gpsimd, i)
                    issue_out(gpsimd, i, out_chunks - 1)

        if pool_chunk >= 0 and not trig_out:
            # One mid-stream input chunk goes through gpsimd's software DGE:
            # its ~1.1us Q7 descriptor generation runs on the otherwise-idle
            # Pool engine, in parallel with the shared HWDGE unit that
            # serializes the SP/Act-issued chunks at ~625ns each.
            @block.gpsimd
            def _(gpsimd):
                for i in range(niter):
                    serial_gate(gpsimd, i)
                    issue_in(gpsimd, i, pool_chunk)

        if trig_out:
            # Output DMAs via gpsimd's SWDGE prepare/trigger split: the
            # ~1us/DMA Q7 descriptor generation happens during the input-DMA
            # head (Pool is otherwise idle), so after the last PSUM copy only
            # the cheap ring-doorbell write + transfer + completion sem remain
            # on the critical path (vs ~1.9us of config+HWDGE+DGE-delay for a
            # plain dma_start). kv_writeback with all-zero ctx indices is a
            # plain transposing SBUF->DRAM write.
            @block.gpsimd
            def _(gpsimd):
                for i in range(niter):
                    serial_gate(gpsimd, i)
                    for c, (a, b) in enumerate(obnds):
                        gpsimd.kv_writeback(
                            out[a:b].rearrange("p (r o) u -> p r o u", o=1),
                            sb_out[:, a * U:b * U].rearrange(
                                "r (o p u) -> r o p u", o=1, p=b - a),
                            sb_idx[:, a:b],
                            prepare_only=True,
                            sem=out_sem[c],
                        ).then_inc(prep_sem, 1)
                    for c, (a, b) in enumerate(obnds):
                        gpsimd.wait_ge(prep_sem, out_chunks * i + c + 1)
                        gpsimd.wait_ge(cp_sem, npair * i + b)
                        gpsimd.trigger_dma(count=1)

        @block.tensor
        def _(tensor):
            if warmup:
                tensor.wait_ge(warm_sem, 1)
            tensor.wait_ge(ps_init_sem, 1)
            for i in range(niter):
                serial_gate(tensor, i)
                if warmup:
                    # Dummy matmuls: sustained PE activity walks the p-state
                    # up (0.65 -> 1.2 -> 2.4 GHz) while input DMAs stream, so
                    # the real matmuls run at full clock even in a cold call.
                    # Gated by the serial chain above so each serial iteration
                    # pays for its own ramp, like a real cold call would.
                    for _ in range(warmup):
                        tensor.matmul(
                            ps_warm[:, :], sb_warm[:, 0:128], sb_warm[:, :],
                            start=True, stop=True,
                        )
                s = i % NSET
                for j in range(EPC):
                    p, half = j // 2, j % 2
                    if j == 0 or echunk[j] != echunk[j - 1]:
                        tensor.wait_ge(in_sem[echunk[j]], 16 * (i + 1))
                    if i >= 1 and half == 0:
                        # pair bank p was copied out during iter i-1
                        tensor.wait_ge(cp_sem, npair * (i - 1) + p + 1)
                    for k in range(2):
                        mm = tensor.matmul(
                            ps[p][half * 64:half * 64 + cap, 0:U],
                            xt_ap(s, j, k),
                            w_ap(s, j, k),
                            start=(k == 0),
                            stop=(k == 1),
                        )
                    mm.then_inc(mm_sem, 1)

        @block.vector
        def _(vector):
            for i in range(niter):
                for p in range(npair):
                    vector.wait_ge(mm_sem, 8 * i + 2 * p + 2)
                    if i >= 1:
                        # sb_out chunk was DMAed out during iter i-1
                        vector.wait_ge(out_sem[ochunk[p]], 16 * i)
                    vector.tensor_copy(
                        sb_out[:, p * U:(p + 1) * U], ps[p][:, 0:U]
                    ).then_inc(cp_sem, 1)

    return nc


def _route(content_idx: np.ndarray, x: np.ndarray, cap: int):
    """Sort samples by expert; compute per-core padded x^T shards."""
    idx = content_idx.reshape(-1).astype(np.int64)
    order = np.argsort(idx, kind="stable")
    e_sorted = idx[order]
    counts = np.bincount(idx, minlength=C)
    while counts.max() > cap:
        cap += 16
    start = np.zeros(C, dtype=np.int64)
    start[1:] = np.cumsum(counts)[:-1]
    slot = np.arange(B) - start[e_sorted]
    core = e_sorted // EPC
    col = (e_sorted % EPC) * cap + slot

    xt_all = np.zeros((NCORES, D, EPC * cap), dtype=np.float32)
    xt_all[core, :, col] = x[order]
    return cap, order, core, col, xt_all


def _unshard(outs: np.ndarray, order, core, col, cap: int) -> np.ndarray:
    """Scatter per-core padded device output back to original sample order.

    outs: (NCORES, npair, 128, U) bf16; expert pair p holds local expert 2p
    at rows 0:cap and 2p+1 at rows 64:64+cap.
    """
    scale = W_SCALE if WDT.startswith("fp8") else 1.0
    out_full = np.empty((B, U), dtype=np.float32)
    jl = col // cap          # local expert index
    slot = col % cap
    out_full[order] = outs[core, jl // 2, (jl % 2) * 64 + slot, :].astype(np.float32)
    if scale != 1.0:
        out_full /= scale
    return out_full


def _make_in_maps(xt_all: np.ndarray, kernel_w: np.ndarray):
    """Build the packed per-core input byte image [128, NB]."""
    bf16 = ml_dtypes.bfloat16
    cap = xt_all.shape[2] // EPC
    xeb, wb, eb, nb = _layout(cap, WDT)
    if WDT == "bf16":
        wdev = kernel_w.reshape(NCORES, EPC, D, U).astype(bf16)
    elif WDT == "fp8e3":
        wdev = (kernel_w.reshape(NCORES, EPC, D, U) * W_SCALE).astype(
            ml_dtypes.float8_e3m4)
    elif WDT == "fp8e4":
        wdev = (kernel_w.reshape(NCORES, EPC, D, U) * W_SCALE).astype(
            ml_dtypes.float8_e4m3)
    else:
        raise ValueError(WDT)

    # per expert j: [xt_k0 | xt_k1 | w_k0 | w_k1], all indexed by partition p
    img = np.empty((NCORES, 128, EPC, eb), dtype=np.uint8)
    xt16 = xt_all.astype(bf16)                       # [NC, 256, EPC*cap]
    # [c, k, p, e, cap] -> [c, p, e, k, cap]
    xtb = xt16.reshape(NCORES, 2, 128, EPC, cap).transpose(0, 2, 3, 1, 4)
    img[:, :, :, :xeb] = np.ascontiguousarray(xtb).view(np.uint8).reshape(
        NCORES, 128, EPC, xeb)
    # [c, e, k, p, u] -> [c, p, e, k, u]
    wkb = wdev.reshape(NCORES, EPC, 2, 128, U).transpose(0, 3, 1, 2, 4)
    img[:, :, :, xeb:] = np.ascontiguousarray(wkb).view(np.uint8).reshape(
        NCORES, 128, EPC, wb)
    img = img.reshape(NCORES, 128, nb)
    return [{"inp": img[c]} for c in range(NCORES)]


def kernel(content_idx: np.ndarray, x: np.ndarray, kernel: np.ndarray) -> np.ndarray:
    from concourse.bass_utils import run_bass_kernel_spmd

    cap, order, core, col, xt_all = _route(content_idx, x, CAP)
    if cap > 64:
        # Pathologically skewed routing (an expert holds >64 samples) can't
        # use the static pair-packed program (PE tile offsets allow only
        # {0,64}). Unreachable for the fixed-seed problem data; fall back to
        # a host computation to stay correct.
        idx = content_idx.reshape(-1).astype(np.int64)
        return np.einsum("bd,bdu->bu", x.astype(np.float32),
                         kernel.astype(np.float32)[idx]).astype(np.float32)

    key = (cap, 1)
    if key not in _prog_cache:
        _prog_cache[key] = _build_program(cap, 1)
    nc = _prog_cache[key]

    in_maps = _make_in_maps(xt_all, kernel)
    res = run_bass_kernel_spmd(nc, in_maps, list(range(NCORES)))
    outs = np.stack([np.asarray(res.results[c]["out"]) for c in range(NCORES)])
    return _unshard(outs, order, core, col, cap)


# revision 21
# speedup vs baseline: 2.1375x; 1.0787x over previous
"""MoE routed matmul on 8 NeuronCores (Trainium2, Bass).

Problem: out[b] = x[b] @ W[idx[b]]  with  x:(2048,256), W:(64,256,256),
idx:(2048,1) int32.

Strategy: expert-parallel. Experts (contexts) are sharded 8-per-core.
The host routes samples to the core that owns their expert (this is the
all-to-all, done during input sharding), padding each expert's sample
group to a fixed capacity CAP so the SPMD device program is fully
static. Each core then does 8 dense (CAP x 256) @ (256 x 256) matmuls —
weights are read from HBM exactly once across the whole device, which is
what the memory-bound roofline wants. The host scatters the device
output back to the original sample order.

Performance structure (vs the 15.6us f32 baseline, which was PE-bound:
fp32 streams at 4 cycles/row at the 1.2GHz mid p-state):
  - x and the output travel as bf16, weights as fp8 e3m4 pre-scaled by
    W_SCALE (rel err 1.2e-2 on the problem data vs the 2e-2 gate;
    bf16-everywhere is 3.0e-3 and one flag away). PE streams the moving
    operand at 1 cycle/row for both.
  - ALL device inputs are packed on the host into one partition-major
    byte image [128, NB] (xt bf16, then each expert's W). The device
    DMAs it in a few large fully-contiguous column chunks — DMA configs
    (~650ns) and the shared HWDGE descriptor-gen unit (~630ns/DMA) are
    the serial bottleneck, not bytes, so fewer+bigger beats many+small.
    Matmul operands are bitcast views into the image.
  - DMA issue only on SP/Activation (HWDGE). gpsimd's software DGE costs
    ~1.1us/DMA on the Q7 cores; DVE can't issue DMAs at all.
  - CAP=48 (max per-expert count for this data is 45): expert pairs sit
    in one PSUM bank at partition offsets 0/64 (PE tile positions allow
    only {0,64}), copied out full-128-wide; the host skips the pad rows.
  - warmup matmuls on zeroed SBUF bridge the PE p-state ramp
    (0.65 -> 1.2 -> 2.4 GHz after 3us continuously busy) across the
    input-DMA head so the real matmuls run at full clock.

Device program per core (raw Bass, manual semaphores):
  sync   : DMA input-image chunk 0 (xt + first experts), last out chunk
  scalar : DMA remaining input chunks, first out chunk(s)
  tensor : warmup matmuls; per expert, 2 accumulating matmuls (K=256
           split in 2) into a PSUM half-bank at offset 0/64
  vector : PSUM -> SBUF copy per expert pair, f32 -> bf16

niter > 1 replicates the body with double-buffered inputs and WAR
semaphore chaining — used by the benchmark harness to measure
steady-state per-iteration HW time via wall-clock slope. serial=True
chains every engine's iteration i behind iteration i-1's output-DMA
completion semaphores, so each iteration is a faithful isolated cold
call (no cross-iteration overlap; warmup matmuls are gated the same way
and their cost is included).
"""

import numpy as np
from contextlib import ExitStack

import ml_dtypes

B, D, U, C = 2048, 256, 256, 64
NCORES = 8
EPC = C // NCORES  # experts per core
CAP = 48           # per-expert sample capacity (padded); data max is 45

WDT = "fp8e3"      # device weight dtype: "bf16" | "fp8e3"
W_SCALE = 256.0    # fp8 weights are pre-scaled by this; host divides out

# input-image chunk split: experts per input DMA (chunk 0 also carries xt)
INSPLIT = (1, 3, 4)
OUT_CHUNKS = 2
WARMUP = 6

_prog_cache: dict = {}


def _wsize(wdt: str) -> int:
    return 2 if wdt == "bf16" else 1


def _layout(cap: int, wdt: str):
    """Byte layout of the packed input image (per partition).

    Per expert j (interleaved so any expert range is byte-contiguous):
      [xt_k0 (cap bf16) | xt_k1 (cap bf16) | w_k0 (U wdt) | w_k1 (U wdt)]
    """
    xeb = 2 * cap * 2               # xt bytes per expert (both K-chunks)
    wb = 2 * U * _wsize(wdt)        # W bytes per expert (both K-chunks)
    eb = xeb + wb
    nb = EPC * eb
    return xeb, wb, eb, nb


def _build_program(cap: int, niter: int = 1, serial: bool = False,
                   wdt: str = WDT, insplit=INSPLIT, out_chunks: int = OUT_CHUNKS,
                   warmup: int = WARMUP, trig_out: bool = False,
                   pool_chunk: int = -1, tail_eng: str = "sp"):
    import concourse.bass as bass
    from concourse import mybir
    from concourse.bass import compact_to_ranges

    f32 = mybir.dt.float32
    bf16 = mybir.dt.bfloat16
    u8 = mybir.dt.uint8
    i32 = mybir.dt.int32
    wdtype = {"bf16": bf16, "fp8e3": mybir.dt.float8e3,
              "fp8e4": mybir.dt.float8e4}[wdt]
    assert cap % 16 == 0 and 16 <= cap <= 64
    assert sum(insplit) == EPC
    npair = EPC // 2
    osplit = ((npair // out_chunks,) * out_chunks
              if isinstance(out_chunks, int) else tuple(out_chunks))
    assert sum(osplit) == npair
    # pair range [oa, ob) per output chunk + chunk of each pair
    obnds, oa = [], 0
    for n in osplit:
        obnds.append((oa, oa + n))
        oa += n
    ochunk = {p: c for c, (a, b) in enumerate(obnds) for p in range(a, b)}
    out_chunks = len(osplit)
    xeb, wb, eb, nb = _layout(cap, wdt)

    # input chunk column ranges [a, b) in the byte image + expert coverage
    chunks = []
    e0 = 0
    for ne in insplit:
        chunks.append((e0 * eb, (e0 + ne) * eb, e0, e0 + ne))
        e0 += ne
    echunk = {}
    for ci, (_, _, ea, ebnd) in enumerate(chunks):
        for j in range(ea, ebnd):
            echunk[j] = ci
    nchunk = len(chunks)

    nc = bass.Bass()
    inp = nc.declare_dram_parameter("inp", [128, nb], u8, isOutput=False)
    # per pair: expert 2p at rows 0:cap, expert 2p+1 at rows 64:64+cap
    out = nc.declare_dram_parameter("out", [npair, 128, U], bf16, isOutput=True)

    NSET = 2 if niter > 1 else 1

    with ExitStack() as ctx:
        sb_in = [
            ctx.enter_context(nc.sbuf_tensor(f"sb_in{s}", [128, nb], u8))
            for s in range(NSET)
        ]
        # one contiguous out staging tensor so a chunk of pairs goes out in
        # one DMA: pair p lives at columns [p*U, (p+1)*U)
        sb_out = ctx.enter_context(nc.sbuf_tensor("sb_out", [128, npair * U], bf16))
        # one full PSUM bank per expert pair
        ps = [
            ctx.enter_context(nc.psum_tensor(f"ps{p}", [128, 512], f32))
            for p in range(npair)
        ]
        if warmup:
            sb_warm = ctx.enter_context(nc.sbuf_tensor("sb_warm", [128, 512], bf16))
            ps_warm = ctx.enter_context(nc.psum_tensor("ps_warm", [128, 512], f32))
        if trig_out:
            # zero ctx indices for the kv_writeback-shaped output DMA
            sb_idx = ctx.enter_context(nc.sbuf_tensor("sb_idx", [128, npair], i32))

        # Dedicated sems per buffer group: a wait threshold on a sem that
        # counts several in-flight DMAs is unsound (a DMA's +16 completion
        # is split +1 across 16 SDMA engines, so a later DMA's increments
        # can satisfy an earlier DMA's threshold while it still has a
        # straggler engine). One sem per buffer makes thresholds exact.
        warm_sem = ctx.enter_context(nc.semaphore("warm_sem"))
        ps_init_sem = ctx.enter_context(nc.semaphore("ps_init_sem"))
        in_sem = [ctx.enter_context(nc.semaphore(f"in_sem{t}"))
                  for t in range(nchunk)]
        mm_sem = ctx.enter_context(nc.semaphore("mm_sem"))
        cp_sem = ctx.enter_context(nc.semaphore("cp_sem"))
        out_sem = [ctx.enter_context(nc.semaphore(f"out_sem{c}"))
                   for c in range(out_chunks)]
        if trig_out:
            prep_sem = ctx.enter_context(nc.semaphore("prep_sem"))

        # Semaphores are NOT cleared when a loaded NEFF is re-executed, so
        # absolute wait thresholds would be stale on the second run. Clear
        # the whole kernel sem range up front (same preamble the BIR
        # lowering path emits), then a pseudo-sync barrier keeps every
        # engine parked until the clears retire.
        for sem_range in compact_to_ranges(
            [s for s in nc._kernel_sem_range if s not in nc.barrier_sems]
        ):
            nc.gpsimd.dma_reset(sem_range)
            nc.gpsimd.sem_clear(sem_range)
        nc._nrt_pseudo_barrier()
        if warmup:
            nc.gpsimd.memset(sb_warm[:, :], 0.0)
            nc.gpsimd.sem_inc(warm_sem, 1)
        if trig_out:
            from concourse import library_config
            nc.gpsimd.load_library(library_config.attn)
            nc.gpsimd.memset(sb_idx[:, :], 0)
        # One-time zero of the PSUM pair banks: rows outside the expert
        # capacity (cap:64, 64+cap:128) are never written by matmuls but ARE
        # copied/DMAed (full-128 ops beat garbage-skipping APs); the host
        # ignores them. Matmuls only rewrite their own rows, so a single
        # preamble memset keeps the pad rows finite forever.
        for p in range(npair):
            nc.vector.memset(ps[p][:, :], 0.0)
        nc.vector.sem_inc(ps_init_sem, 1)

        block = ctx.enter_context(nc.Block())

        def xt_ap(s, j, k):
            a = j * eb + k * cap * 2
            return sb_in[s][:, a:a + cap * 2].bitcast(bf16)

        def w_ap(s, j, k):
            a = j * eb + xeb + k * (wb // 2)
            return sb_in[s][:, a:a + wb // 2].bitcast(wdtype)

        # out chunk c: DRAM [nc_pairs, 128, U] <- SBUF [128, nc_pairs, U]
        out_r = [
            out[a:b].rearrange("p r u -> r p u")
            for a, b in obnds
        ]

        def serial_gate(eng, i):
            if serial and i >= 1:
                for c in range(out_chunks):
                    eng.wait_ge(out_sem[c], 16 * i)

        def issue_in(eng, i, ci):
            s = i % NSET
            a, b, ea, ebnd = chunks[ci]
            if i >= 2:
                # chunk ci of set s was read by its own experts' matmuls of
                # iter i-2 (the chunk carries those experts' xt AND W)
                eng.wait_ge(mm_sem, 8 * (i - 2) + ebnd)
            eng.dma_start(sb_in[s][:, a:b], inp[:, a:b]).then_inc(in_sem[ci], 16)

        def issue_out(eng, i, c):
            a, b = obnds[c]
            eng.wait_ge(cp_sem, npair * i + b)
            eng.dma_start(
                out_r[c],
                sb_out[:, a * U:b * U].rearrange("r (p u) -> r p u", p=b - a),
            ).then_inc(out_sem[c], 16)

        @block.sync
        def _(sync):
            for i in range(niter):
                serial_gate(sync, i)
                issue_in(sync, i, 0)
                if not trig_out and tail_eng == "sp":
                    issue_out(sync, i, out_chunks - 1)
            for c in range(out_chunks):
                sync.wait_ge(out_sem[c], 16 * niter)

        @block.scalar
        def _(scalar):
            for i in range(niter):
                serial_gate(scalar, i)
                for ci in range(1, nchunk):
                    if ci != pool_chunk:
                        issue_in(scalar, i, ci)
                if not trig_out:
                    for c in range(out_chunks - 1):
                        issue_out(scalar, i, c)

        if not trig_out and tail_eng == "pool":
            # The last out chunk via gpsimd software DGE: from the final
            # copy's semaphore, Pool's chain (~60ns dispatch + ~1.1us Q7
            # descriptor gen + transfer + sem) undercuts the HWDGE chain
            # (config 650 + gen 625 + DGE delay 650 + transfer + sem).
            @block.gpsimd
            def _(gpsimd):
                for i in range(niter):
                    serial_gate(gpsimd, i)
                    issue_out(gpsimd, i, out_chunks - 1)

        if pool_chunk >= 0 and not trig_out:
            # One mid-stream input chunk goes through gpsimd's software DGE:
            # its ~1.1us Q7 descriptor generation runs on the otherwise-idle
            # Pool engine, in parallel with the shared HWDGE unit that
            # serializes the SP/Act-issued chunks at ~625ns each.
            @block.gpsimd
            def _(gpsimd):
                for i in range(niter):
                    serial_gate(gpsimd, i)
                    issue_in(gpsimd, i, pool_chunk)

        if trig_out:
            # Output DMAs via gpsimd's SWDGE prepare/trigger split: the
            # ~1us/DMA Q7 descriptor generation happens during the input-DMA
            # head (Pool is otherwise idle), so after the last PSUM copy only
            # the cheap ring-doorbell write + transfer + completion sem remain
            # on the critical path (vs ~1.9us of config+HWDGE+DGE-delay for a
            # plain dma_start). kv_writeback with all-zero ctx indices is a
            # plain transposing SBUF->DRAM write.
            @block.gpsimd
            def _(gpsimd):
                for i in range(niter):
                    serial_gate(gpsimd, i)
                    for c, (a, b) in enumerate(obnds):
                        gpsimd.kv_writeback(
                            out[a:b].rearrange("p (r o) u -> p r o u", o=1),
                            sb_out[:, a * U:b * U].rearrange(
                                "r (o p u) -> r o p u", o=1, p=b - a),
                            sb_idx[:, a:b],
                            prepare_only=True,
                            sem=out_sem[c],
                        ).then_inc(prep_sem, 1)
                    for c, (a, b) in enumerate(obnds):
                        gpsimd.wait_ge(prep_sem, out_chunks * i + c + 1)
                        gpsimd.wait_ge(cp_sem, npair * i + b)
                        gpsimd.trigger_dma(count=1)

        @block.tensor
        def _(tensor):
            if warmup:
                tensor.wait_ge(warm_sem, 1)
            tensor.wait_ge(ps_init_sem, 1)
            for i in range(niter):
                serial_gate(tensor, i)
                if warmup:
                    # Dummy matmuls: sustained PE activity walks the p-state
                    # up (0.65 -> 1.2 -> 2.4 GHz) while input DMAs stream, so
                    # the real matmuls run at full clock even in a cold call.
                    # Gated by the serial chain above so each serial iteration
                    # pays for its own ramp, like a real cold call would.
                    for _ in range(warmup):
                        tensor.matmul(
                            ps_warm[:, :], sb_warm[:, 0:128], sb_warm[:, :],
                            start=True, stop=True,
                        )
                s = i % NSET
                for j in range(EPC):
                    p, half = j // 2, j % 2
                    if j == 0 or echunk[j] != echunk[j - 1]:
                        tensor.wait_ge(in_sem[echunk[j]], 16 * (i + 1))
                    if i >= 1 and half == 0:
                        # pair bank p was copied out during iter i-1
                        tensor.wait_ge(cp_sem, npair * (i - 1) + p + 1)
                    for k in range(2):
                        mm = tensor.matmul(
                            ps[p][half * 64:half * 64 + cap, 0:U],
                            xt_ap(s, j, k),
                            w_ap(s, j, k),
                            start=(k == 0),
                            stop=(k == 1),
                        )
                    mm.then_inc(mm_sem, 1)

        @block.vector
        def _(vector):
            for i in range(niter):
                for p in range(npair):
                    vector.wait_ge(mm_sem, 8 * i + 2 * p + 2)
                    if i >= 1:
                        # sb_out chunk was DMAed out during iter i-1
                        vector.wait_ge(out_sem[ochunk[p]], 16 * i)
                    vector.tensor_copy(
                        sb_out[:, p * U:(p + 1) * U], ps[p][:, 0:U]
                    ).then_inc(cp_sem, 1)

    return nc


def _route(content_idx: np.ndarray, x: np.ndarray, cap: int):
    """Sort samples by expert; compute per-core padded x^T shards."""
    idx = content_idx.reshape(-1).astype(np.int64)
    order = np.argsort(idx, kind="stable")
    e_sorted = idx[order]
    counts = np.bincount(idx, minlength=C)
    while counts.max() > cap:
        cap += 16
    start = np.zeros(C, dtype=np.int64)
    start[1:] = np.cumsum(counts)[:-1]
    slot = np.arange(B) - start[e_sorted]
    core = e_sorted // EPC
    col = (e_sorted % EPC) * cap + slot

    xt_all = np.zeros((NCORES, D, EPC * cap), dtype=np.float32)
    xt_all[core, :, col] = x[order]
    return cap, order, core, col, xt_all


def _unshard(outs: np.ndarray, order, core, col, cap: int) -> np.ndarray:
    """Scatter per-core padded device output back to original sample order.

    outs: (NCORES, npair, 128, U) bf16; expert pair p holds local expert 2p
    at rows 0:cap and 2p+1 at rows 64:64+cap.
    """
    scale = W_SCALE if WDT.startswith("fp8") else 1.0
    out_full = np.empty((B, U), dtype=np.float32)
    jl = col // cap          # local expert index
    slot = col % cap
    out_full[order] = outs[core, jl // 2, (jl % 2) * 64 + slot, :].astype(np.float32)
    if scale != 1.0:
        out_full /= scale
    return out_full


def _make_in_maps(xt_all: np.ndarray, kernel_w: np.ndarray):
    """Build the packed per-core input byte image [128, NB]."""
    bf16 = ml_dtypes.bfloat16
    cap = xt_all.shape[2] // EPC
    xeb, wb, eb, nb = _layout(cap, WDT)
    if WDT == "bf16":
        wdev = kernel_w.reshape(NCORES, EPC, D, U).astype(bf16)
    elif WDT == "fp8e3":
        wdev = (kernel_w.reshape(NCORES, EPC, D, U) * W_SCALE).astype(
            ml_dtypes.float8_e3m4)
    elif WDT == "fp8e4":
        wdev = (kernel_w.reshape(NCORES, EPC, D, U) * W_SCALE).astype(
            ml_dtypes.float8_e4m3)
    else:
        raise ValueError(WDT)

    # per expert j: [xt_k0 | xt_k1 | w_k0 | w_k1], all indexed by partition p
    img = np.empty((NCORES, 128, EPC, eb), dtype=np.uint8)
    xt16 = xt_all.astype(bf16)                       # [NC, 256, EPC*cap]
    # [c, k, p, e, cap] -> [c, p, e, k, cap]
    xtb = xt16.reshape(NCORES, 2, 128, EPC, cap).transpose(0, 2, 3, 1, 4)
    img[:, :, :, :xeb] = np.ascontiguousarray(xtb).view(np.uint8).reshape(
        NCORES, 128, EPC, xeb)
    # [c, e, k, p, u] -> [c, p, e, k, u]
    wkb = wdev.reshape(NCORES, EPC, 2, 128, U).transpose(0, 3, 1, 2, 4)
    img[:, :, :, xeb:] = np.ascontiguousarray(wkb).view(np.uint8).reshape(
        NCORES, 128, EPC, wb)
    img = img.reshape(NCORES, 128, nb)
    return [{"inp": img[c]} for c in range(NCORES)]


def kernel(content_idx: np.ndarray, x: np.ndarray, kernel: np.ndarray) -> np.ndarray:
    from concourse.bass_utils import run_bass_kernel_spmd

    content_idx = np.asarray(content_idx)
    x = np.asarray(x, dtype=np.float32)
    kernel = np.asarray(kernel, dtype=np.float32)

    cap, order, core, col, xt_all = _route(content_idx, x, CAP)
    if cap > 64:
        # Pathologically skewed routing (an expert holds >64 samples) can't
        # use the static pair-packed program (PE tile offsets allow only
        # {0,64}). Unreachable for the fixed-seed problem data; fall back to
        # a host computation to stay correct.
        idx = content_idx.reshape(-1).astype(np.int64)
        return np.einsum("bd,bdu->bu", x.astype(np.float32),
                         kernel.astype(np.float32)[idx]).astype(np.float32)

    key = (cap, 1)
    if key not in _prog_cache:
        _prog_cache[key] = _build_program(cap, 1)
    nc = _prog_cache[key]

    in_maps = _make_in_maps(xt_all, kernel)
    res = run_bass_kernel_spmd(nc, in_maps, list(range(NCORES)))
    outs = np.stack([np.asarray(res.results[c]["out"]) for c in range(NCORES)])
    return _unshard(outs, order, core, col, cap)
